# revision 19
# baseline (speedup 1.0000x reference)
"""Trainium2 Bass kernel for nn_Double_SSM_Block_Encoder.

Double Mamba (SSM) block encoder over (b=8, c=64, h=64, w=64) inputs.
Sharding: data-parallel over batch, 1 batch element per NeuronCore (8 cores).

Per-core layout: channel-major [channels on partitions, time t = h*64+w on free].
Key mappings:
  - input projection + depthwise causal conv fused into 4 shifted matmuls
    (host folds w_conv into w_in)
  - dt = softplus via Exp then Ln(1+x) (softplus not in ACT tables)
  - per-state decay dA_n = Exp(A[:,n] * dt) via ACT with per-partition scale
  - recurrence h = dA*h + dBx via hardware tensor_tensor_scan (fp32 state)
  - B/C broadcast over channels via PE ones-matmul into PSUM
  - layernorm over channels via PE ones-matmul stats
  - final (b,h,w,2c)->(b,2c,w,h) permute folded into the last ACT write AP
"""
import sys, types, contextlib, ctypes
sys.path.insert(0, "/opt/trn_rl_repo")
import numpy as np

# ---- axon NTFF profile hook shim (image's antenv lacks axon_hooks) ----------
def _make_ntff_hook(so_path="/opt/axon/libaxon_pjrt.so"):
    try:
        lib = ctypes.CDLL(so_path)
    except OSError:
        return None
    if not hasattr(lib, "axon_start_nrt_profile"):
        return None
    lib.axon_start_nrt_profile.argtypes = [ctypes.POINTER(ctypes.c_int64), ctypes.c_size_t]
    lib.axon_start_nrt_profile.restype = ctypes.c_int64
    lib.axon_stop_nrt_profile.argtypes = [ctypes.c_char_p]
    lib.axon_stop_nrt_profile.restype = ctypes.c_int64

    @contextlib.contextmanager
    def _hook(output_dir, device_ids):
        import jax
        jax.devices()
        if device_ids:
            ids = (ctypes.c_int64 * len(device_ids))(*device_ids)
            rc = lib.axon_start_nrt_profile(ids, len(device_ids))
        else:
            rc = lib.axon_start_nrt_profile(None, 0)
        if rc != 0:
            raise RuntimeError(f"axon_start_nrt_profile rc={rc}")
        try:
            yield
        finally:
            rc = lib.axon_stop_nrt_profile(str(output_dir).encode())
            if rc != 0:
                raise RuntimeError(f"axon_stop_nrt_profile rc={rc}")
    return _hook

if "antenv.axon_hooks" not in sys.modules:
    _hooks_mod = types.ModuleType("antenv.axon_hooks")
    _HOOK = _make_ntff_hook()
    _hooks_mod.get_axon_ntff_profile_hook = lambda: _HOOK
    _hooks_mod.set_axon_ntff_profile_hook = lambda h: None
    sys.modules["antenv.axon_hooks"] = _hooks_mod

import concourse.bass as bass
import concourse.tile as tile
from concourse import mybir
from concourse import bass_utils
bass_utils.upload_artifacts = lambda tmpdir: tmpdir  # no S3 in this container
from contextlib import ExitStack

F32 = mybir.dt.float32
BF16 = mybir.dt.bfloat16
F16 = mybir.dt.float16
AF = mybir.ActivationFunctionType
OP = mybir.AluOpType

NCORES = 8
CIN = 64        # model channels in
D = 128         # d_inner
NST = 16        # d_state
RANK = 4        # dt_rank
KCONV = 4
L = 4096
T = 512         # time tile for PSUM-bound ops
NT = L // T
CH = 1024       # n-loop chunk length
NCH = L // CH


def _legalize_sync_waits(nc):
    """Walrus codegen allows only one inline sync-wait per compute
    instruction; hoist surplus waits onto a preceding same-engine Drain."""
    SAFE = set()
    for f in nc.m.functions:
        for blk in f.blocks:
            insts = blk.instructions
            i = 0
            while i < len(insts):
                inst = insts[i]
                si = inst.sync_info
                if (si is not None and si.on_wait and len(si.on_wait) > 1
                        and inst.opcode not in SAFE):
                    waits = list(si.on_wait)
                    for w in waits[:-1]:
                        d = mybir.InstDrain(
                            name=nc.get_next_instruction_name(),
                            ins=[], outs=[], bass_is_fusable=False)
                        d.engine = inst.engine
                        d.sync_info = mybir.SyncInfo(on_wait=[w], on_update=[])
                        insts.insert(i, d)
                        i += 1
                    inst.sync_info = mybir.SyncInfo(
                        on_wait=[waits[-1]], on_update=list(si.on_update))
                    i += 1
                else:
                    i += 1


SIM_SAFE = False  # emit Silu as Identity+Sigmoid+mul so CoreSim can run it


def _emit_silu(nc, nlp, out_sl, in_ps, bias, blk_i, j, which):
    if not SIM_SAFE:
        if bias is None:
            nc.scalar.activation(out_sl, in_ps, AF.Silu)
        else:
            nc.scalar.activation(out_sl, in_ps, AF.Silu, bias=bias)
        return
    v = nlp.tile(list(in_ps.shape), F32, tag="lnt", name=f"sv_{which}_{blk_i}_{j}")
    if bias is None:
        nc.scalar.activation(v[:], in_ps, AF.Identity)
    else:
        nc.scalar.activation(v[:], in_ps, AF.Identity, bias=bias)
    s = nlp.tile(list(in_ps.shape), F32, tag="lnt2", name=f"ss_{which}_{blk_i}_{j}")
    nc.scalar.activation(s[:], v[:], AF.Sigmoid)
    nc.vector.tensor_mul(out_sl, v[:], s[:])


def _emit_block(nc, tc, ctx, pools, xpad, P, blk_i, out_final=None):
    """Emit one mamba block + layernorm + relu.

    xpad: SBUF [CIN, 3+L] fp32, first 3 cols zero.
    Returns x2pad tile (next block input) if out_final is None, else writes
    the permuted result into out_final.
    """
    const, big, nlp, psA, psB, psP = pools
    COUT = P["wout"].shape[1]   # 64 for block1, 128 for block2

    # ---- stage 1: xz matmuls (conv folded), silu ----
    xc = big.tile([D, L], F32, tag="xc", name=f"xc_{blk_i}")
    zs = big.tile([D, L], F16, tag="zs", name=f"zs_{blk_i}")
    for j in range(NT):
        ps_xc = psA.tile([D, T], F32, tag="mm", name=f"psxc_{blk_i}_{j}")
        for k in range(KCONV):
            nc.tensor.matmul(ps_xc[:], P["wk"][k][:], xpad[:, j*T + k : j*T + k + T],
                             start=(k == 0), stop=(k == KCONV - 1))
        _emit_silu(nc, nlp, xc[:, j*T:(j+1)*T], ps_xc[:], P["bconv"][:], blk_i, j, "xc")
        ps_z = psA.tile([D, T], F32, tag="mm", name=f"psz_{blk_i}_{j}")
        nc.tensor.matmul(ps_z[:], P["wz"][:], xpad[:, 3 + j*T : 3 + (j+1)*T],
                         start=True, stop=True)
        _emit_silu(nc, nlp, zs[:, j*T:(j+1)*T], ps_z[:], None, blk_i, j, "z")

    # ---- stage 2: proj = w_x^T xc -> dtr(4) B(16) C(16) rows ----
    proj = big.tile([RANK + 2*NST, L], F32, tag="proj", name=f"proj_{blk_i}")
    for j in range(NT):
        ps_p = psP.tile([RANK + 2*NST, T], F32, tag="proj", name=f"psp_{blk_i}_{j}")
        nc.tensor.matmul(ps_p[:], P["wx"][:], xc[:, j*T:(j+1)*T], start=True, stop=True)
        nc.scalar.copy(proj[:, j*T:(j+1)*T], ps_p[:])

    # ---- stage 3: dt = softplus(w_dt^T dtr + b_dt) = Ln(1 + Exp(.)) ----
    dt = big.tile([D, L], F32, tag="dt", name=f"dt_{blk_i}")
    for j in range(NT):
        ps_d = psA.tile([D, T], F32, tag="mm", name=f"psd_{blk_i}_{j}")
        nc.tensor.matmul(ps_d[:], P["wdt"][:], proj[0:RANK, j*T:(j+1)*T],
                         start=True, stop=True)
        e_t = nlp.tile([D, T], F32, tag="lnt", name=f"spe_{blk_i}_{j}")
        nc.scalar.activation(e_t[:], ps_d[:], AF.Exp, bias=P["bdt"][:])
        nc.scalar.activation(dt[:, j*T:(j+1)*T], e_t[:], AF.Ln, bias=const["one_d"][:])

    # ---- stage 4: dtxc = dt * xc; y_acc init = xc * D (skip path) ----
    dtxc = big.tile([D, L], F32, tag="dtxc", name=f"dtxc_{blk_i}")
    nc.vector.tensor_mul(dtxc[:], dt[:], xc[:])
    y_acc = big.tile([D, L], F16, tag="yacc", name=f"yacc_{blk_i}")
    nc.vector.tensor_scalar(y_acc[:], xc[:], P["D"][:], None, OP.mult)

    # ---- stage 5: per-state scan & accumulate y ----
    TBC = T  # bcast matmul tile
    for n in range(NST):
        brow = big.tile([1, L], F32, tag="brow", bufs=2, name=f"brow_{blk_i}_{n}")
        nc.sync.dma_start(brow[:], proj[RANK + n : RANK + n + 1, :])
        crow = big.tile([1, L], F32, tag="brow", bufs=2, name=f"crow_{blk_i}_{n}")
        nc.sync.dma_start(crow[:], proj[RANK + NST + n : RANK + NST + n + 1, :])
        hprev = None
        for c in range(NCH):
            c0 = c * CH
            dA = nlp.tile([D, CH], F32, tag="dA", name=f"dA_{blk_i}_{n}_{c}")
            nc.scalar.activation(dA[:], dt[:, c0:c0+CH], AF.Exp, scale=P["A"][:, n:n+1])
            dbx = nlp.tile([D, CH], F32, tag="dbx", name=f"dbx_{blk_i}_{n}_{c}")
            for j in range(CH // TBC):
                t0 = c0 + j * TBC
                ps_b = psB.tile([D, TBC], F32, tag="bc", name=f"psb_{blk_i}_{n}_{c}_{j}")
                nc.tensor.matmul(ps_b[:], const["ones1"][:], brow[0:1, t0:t0+TBC],
                                 start=True, stop=True)
                nc.vector.tensor_mul(dbx[:, j*TBC:(j+1)*TBC],
                                     dtxc[:, t0:t0+TBC], ps_b[:])
            h = nlp.tile([D, CH], F16, tag="h", name=f"h_{blk_i}_{n}_{c}")
            init = 0.0 if c == 0 else hprev[:, CH-1:CH]
            nc.vector.tensor_tensor_scan(h[:], dA[:], dbx[:], init, OP.mult, OP.add)
            hC = nlp.tile([D, CH], F16, tag="dbx", name=f"hC_{blk_i}_{n}_{c}")
            for j in range(CH // TBC):
                t0 = c0 + j * TBC
                ps_c = psB.tile([D, TBC], F32, tag="bc", name=f"psc_{blk_i}_{n}_{c}_{j}")
                nc.tensor.matmul(ps_c[:], const["ones1"][:], crow[0:1, t0:t0+TBC],
                                 start=True, stop=True)
                nc.vector.tensor_mul(hC[:, j*TBC:(j+1)*TBC],
                                     h[:, j*TBC:(j+1)*TBC], ps_c[:])
            nc.vector.tensor_add(y_acc[:, c0:c0+CH], y_acc[:, c0:c0+CH], hC[:])
            hprev = h

    # ---- stage 6: y = y_acc * zs (in place) ----
    nc.vector.tensor_mul(y_acc[:], y_acc[:], zs[:])

    # ---- stage 7: out matmul + layernorm stats ----
    y1 = big.tile([COUT, L], F32, tag="proj", name=f"y1_{blk_i}")
    mu = big.tile([1, L], F32, tag="dt", name=f"mu_{blk_i}")
    musq = big.tile([1, L], F32, tag="musq", name=f"musq_{blk_i}")
    for j in range(NT):
        ps_y = psA.tile([COUT, T], F32, tag="mm", name=f"psy_{blk_i}_{j}")
        nc.tensor.matmul(ps_y[:], P["wout"][:], y_acc[:, j*T:(j+1)*T],
                         start=True, stop=True)
        nc.scalar.copy(y1[:, j*T:(j+1)*T], ps_y[:])
        y1sq = nlp.tile([COUT, T], F32, tag="lnt", name=f"y1sq_{blk_i}_{j}")
        nc.scalar.activation(y1sq[:], ps_y[:], AF.Square)
        ps_m = psP.tile([1, T], F32, tag="stat", name=f"psm_{blk_i}_{j}")
        nc.tensor.matmul(ps_m[:], P["onesc"][:], y1[:, j*T:(j+1)*T],
                         start=True, stop=True)
        nc.scalar.copy(mu[:, j*T:(j+1)*T], ps_m[:])
        ps_m2 = psP.tile([1, T], F32, tag="stat", name=f"psm2_{blk_i}_{j}")
        nc.tensor.matmul(ps_m2[:], P["onesc"][:], y1sq[:], start=True, stop=True)
        nc.scalar.copy(musq[:, j*T:(j+1)*T], ps_m2[:])
    # var = musq - mu^2 (in place), rstd = exp(-0.5*ln(var+eps)) (in place)
    for j in range(NT):
        sl = slice(j*T, (j+1)*T)
        ps_s = psP.tile([1, T], F32, tag="stat", name=f"pss_{blk_i}_{j}")
        nc.scalar.activation(ps_s[:], mu[:, sl], AF.Square)
        nc.vector.tensor_sub(musq[:, sl], musq[:, sl], ps_s[:])
        nc.scalar.activation(musq[:, sl], musq[:, sl], AF.Ln, bias=const["eps"][:])
        nc.scalar.activation(musq[:, sl], musq[:, sl], AF.Exp, scale=-0.5)

    # apply: out = relu(g*(y1 - mu)*rstd + b)
    if out_final is None:
        x2pad = big.tile([COUT, 3 + L], F32, tag="xpad", name=f"x2pad_{blk_i}")
        nc.vector.memset(x2pad[:, 0:3], 0.0)
    for j in range(NT):
        ps_mb = psA.tile([COUT, T], F32, tag="mm", name=f"psmb_{blk_i}_{j}")
        nc.tensor.matmul(ps_mb[:], P["onesr"][:], mu[:, j*T:(j+1)*T],
                         start=True, stop=True)
        ps_rb = psA.tile([COUT, T], F32, tag="mm", name=f"psrb_{blk_i}_{j}")
        nc.tensor.matmul(ps_rb[:], P["onesr"][:], musq[:, j*T:(j+1)*T],
                         start=True, stop=True)
        t1 = nlp.tile([COUT, T], F32, tag="lnt", name=f"lnt1_{blk_i}_{j}")
        nc.vector.tensor_sub(t1[:], y1[:, j*T:(j+1)*T], ps_mb[:])
        t2 = nlp.tile([COUT, T], F32, tag="lnt2", name=f"lnt2_{blk_i}_{j}")
        nc.vector.tensor_mul(t2[:], t1[:], ps_rb[:])
        if out_final is None:
            nc.scalar.activation(x2pad[:, 3 + j*T : 3 + (j+1)*T], t2[:], AF.Relu,
                                 bias=P["bln"][:], scale=P["gln"][:])
        else:
            in_v = t2[:].rearrange("p (h w) -> p h w", w=64)
            out_v = out_final[:].rearrange("p (w h) -> p h w", h=64)[:, 8*j:8*(j+1), :]
            nc.scalar.activation(out_v, in_v, AF.Relu,
                                 bias=P["bln"][:], scale=P["gln"][:])
    return None if out_final is not None else x2pad


def build_nc(legalize=True, sim_safe=False):
    global SIM_SAFE
    SIM_SAFE = sim_safe
    nc = bass.Bass("TRN2", debug=False)
    f32 = np.float32

    def din(name, shape, dt=F32):
        return nc.dram_tensor(name, list(shape), dt, kind="ExternalInput")

    x_d = din("x", (CIN, L))
    ins = {}
    for b in (1, 2):
        ins[f"wk{b}"] = [din(f"wk{b}_{k}", (CIN, D)) for k in range(KCONV)]
        ins[f"wz{b}"] = din(f"wz{b}", (CIN, D))
        ins[f"bconv{b}"] = din(f"bconv{b}", (D, 1))
        ins[f"wx{b}"] = din(f"wx{b}", (D, RANK + 2*NST))
        ins[f"wdt{b}"] = din(f"wdt{b}", (RANK, D))
        ins[f"bdt{b}"] = din(f"bdt{b}", (D, 1))
        ins[f"A{b}"] = din(f"A{b}", (D, NST))
        ins[f"D{b}"] = din(f"D{b}", (D, 1))
        cout = CIN if b == 1 else 2 * CIN
        ins[f"wout{b}"] = din(f"wout{b}", (D, cout), F16)
        ins[f"gln{b}"] = din(f"gln{b}", (cout, 1))
        ins[f"bln{b}"] = din(f"bln{b}", (cout, 1))
        ins[f"onesc{b}"] = din(f"onesc{b}", (cout, 1))   # 1/cout for mean
        ins[f"onesr{b}"] = din(f"onesr{b}", (1, cout))   # ones row for bcast
    ins["ones1"] = din("ones1", (1, D))
    ins["one_d"] = din("one_d", (D, 1))
    ins["eps"] = din("eps", (1, 1))
    out_d = nc.dram_tensor("out", [2*CIN, L], F32, kind="ExternalOutput")

    with tile.TileContext(nc) as tc:
        with ExitStack() as ctx:
            cpool = ctx.enter_context(tc.tile_pool(name="const", bufs=1))
            big = ctx.enter_context(tc.tile_pool(name="big", bufs=1))
            nlp = ctx.enter_context(tc.tile_pool(name="nloop", bufs=2))
            psA = ctx.enter_context(tc.tile_pool(name="psA", bufs=2, space="PSUM"))
            psB = ctx.enter_context(tc.tile_pool(name="psB", bufs=4, space="PSUM"))
            psP = ctx.enter_context(tc.tile_pool(name="psP", bufs=1, space="PSUM"))

            def load(name, dram):
                t = cpool.tile(list(dram.shape), dram.dtype, tag=name, name=name)
                nc.sync.dma_start(t[:], dram.ap())
                return t

            const = {"ones1": load("ones1", ins["ones1"]),
                     "one_d": load("one_d", ins["one_d"]),
                     "eps": load("eps", ins["eps"])}
            P = {}
            for b in (1, 2):
                cout = CIN if b == 1 else 2 * CIN
                P[b] = {
                    "wk": [load(f"wk{b}_{k}", ins[f"wk{b}"][k]) for k in range(KCONV)],
                    "wz": load(f"wz{b}", ins[f"wz{b}"]),
                    "bconv": load(f"bconv{b}", ins[f"bconv{b}"]),
                    "wx": load(f"wx{b}", ins[f"wx{b}"]),
                    "wdt": load(f"wdt{b}", ins[f"wdt{b}"]),
                    "bdt": load(f"bdt{b}", ins[f"bdt{b}"]),
                    "A": load(f"A{b}", ins[f"A{b}"]),
                    "D": load(f"D{b}", ins[f"D{b}"]),
                    "wout": load(f"wout{b}", ins[f"wout{b}"]),
                    "gln": load(f"gln{b}", ins[f"gln{b}"]),
                    "bln": load(f"bln{b}", ins[f"bln{b}"]),
                    "onesc": load(f"onesc{b}", ins[f"onesc{b}"]),
                    "onesr": load(f"onesr{b}", ins[f"onesr{b}"]),
                }

            xpad = big.tile([CIN, 3 + L], F32, tag="xpad")
            nc.vector.memset(xpad[:, 0:3], 0.0)
            nc.sync.dma_start(xpad[:, 3:], x_d.ap())

            out_sb = big.tile([2*CIN, L], F32, tag="dtxc")  # dtxc dead by then
            pools = (const, big, nlp, psA, psB, psP)
            x2pad = _emit_block(nc, tc, ctx, pools, xpad, P[1], 1, out_final=None)
            _emit_block(nc, tc, ctx, pools, x2pad, P[2], 2, out_final=out_sb)
            nc.sync.dma_start(out_d.ap(), out_sb[:])

    if legalize:
        _legalize_sync_waits(nc)
    return nc


_NC_CACHE = {}
_LAST_EXEC_NS = {}

def _get_nc():
    if "nc" not in _NC_CACHE:
        _NC_CACHE["nc"] = build_nc()
    return _NC_CACHE["nc"]


def _host_params(inputs):
    """Fold conv into input projection; compute derived tensors."""
    f32 = np.float32
    maps = {}
    for b in (1, 2):
        w_in = np.asarray(inputs[f"w_in{b}"], f32)       # (64, 256)
        w_conv = np.asarray(inputs[f"w_conv{b}"], f32)   # (128, 4)
        cout = CIN if b == 1 else 2 * CIN
        for k in range(KCONV):
            maps[f"wk{b}_{k}"] = np.ascontiguousarray(w_in[:, :D] * w_conv[:, k][None, :])
        maps[f"wz{b}"] = np.ascontiguousarray(w_in[:, D:])
        maps[f"bconv{b}"] = np.asarray(inputs[f"b_conv{b}"], f32).reshape(D, 1)
        maps[f"wx{b}"] = np.asarray(inputs[f"w_x{b}"], f32)
        maps[f"wdt{b}"] = np.asarray(inputs[f"w_dt{b}"], f32)
        maps[f"bdt{b}"] = np.asarray(inputs[f"b_dt{b}"], f32).reshape(D, 1)
        maps[f"A{b}"] = -np.exp(np.asarray(inputs[f"A_log{b}"], f32))
        maps[f"D{b}"] = np.asarray(inputs[f"D{b}"], f32).reshape(D, 1)
        maps[f"wout{b}"] = np.asarray(inputs[f"w_out{b}"], f32).astype(np.float16)
        maps[f"gln{b}"] = np.asarray(inputs[f"g_ln{b}"], f32).reshape(cout, 1)
        maps[f"bln{b}"] = np.asarray(inputs[f"b_ln{b}"], f32).reshape(cout, 1)
        maps[f"onesc{b}"] = np.full((cout, 1), 1.0 / cout, f32)
        maps[f"onesr{b}"] = np.ones((1, cout), f32)
    maps["ones1"] = np.ones((1, D), f32)
    maps["one_d"] = np.ones((D, 1), f32)
    maps["eps"] = np.full((1, 1), 1e-5, f32)
    return maps


def kernel(**inputs):
    nc = _get_nc()
    x = np.asarray(inputs["x"], np.float32)              # (8, 64, 64, 64)
    b, c, hh, ww = x.shape
    params = _host_params(inputs)
    in_maps = []
    for i in range(NCORES):
        m = dict(params)
        m["x"] = np.ascontiguousarray(x[i].reshape(c, hh * ww))
        in_maps.append(m)
    res = bass_utils.run_bass_kernel_spmd(nc, in_maps, core_ids=list(range(NCORES)),
                                          trace=False)
    out = np.stack([res.results[i]["out"] for i in range(NCORES)])
    return out.reshape(b, 2 * c, ww, hh).astype(np.float32)


# revision 21
# speedup vs baseline: 672.3971x; 672.3971x over previous
"""Trainium2 Bass kernel for nn_Double_SSM_Block_Encoder.

Double Mamba (SSM) block encoder over (b=8, c=64, h=64, w=64) inputs.
Sharding: data-parallel over batch, 1 batch element per NeuronCore (8 cores).

Per-core layout: channel-major [channels on partitions, time t = h*64+w on free].
Key mappings:
  - input projection + depthwise causal conv fused into 4 shifted matmuls
    (host folds w_conv into w_in)
  - dt = softplus via Exp then Ln(1+x) (softplus not in ACT tables)
  - per-state decay dA_n = Exp(A[:,n] * dt) via ACT with per-partition scale
  - recurrence h = dA*h + dBx via hardware tensor_tensor_scan (fp32 state)
  - B/C broadcast over channels via PE ones-matmul into PSUM
  - layernorm over channels via PE ones-matmul stats
  - final (b,h,w,2c)->(b,2c,w,h) permute folded into the last ACT write AP
"""
import sys, types, contextlib, ctypes
sys.path.insert(0, "/opt/trn_rl_repo")
import numpy as np

# ---- axon NTFF profile hook shim (image's antenv lacks axon_hooks) ----------
def _make_ntff_hook(so_path="/opt/axon/libaxon_pjrt.so"):
    try:
        lib = ctypes.CDLL(so_path)
    except OSError:
        return None
    if not hasattr(lib, "axon_start_nrt_profile"):
        return None
    lib.axon_start_nrt_profile.argtypes = [ctypes.POINTER(ctypes.c_int64), ctypes.c_size_t]
    lib.axon_start_nrt_profile.restype = ctypes.c_int64
    lib.axon_stop_nrt_profile.argtypes = [ctypes.c_char_p]
    lib.axon_stop_nrt_profile.restype = ctypes.c_int64

    @contextlib.contextmanager
    def _hook(output_dir, device_ids):
        import jax
        jax.devices()
        if device_ids:
            ids = (ctypes.c_int64 * len(device_ids))(*device_ids)
            rc = lib.axon_start_nrt_profile(ids, len(device_ids))
        else:
            rc = lib.axon_start_nrt_profile(None, 0)
        if rc != 0:
            raise RuntimeError(f"axon_start_nrt_profile rc={rc}")
        try:
            yield
        finally:
            rc = lib.axon_stop_nrt_profile(str(output_dir).encode())
            if rc != 0:
                print(f"WARNING: axon_stop_nrt_profile rc={rc} (no NTFF shipped)")
    return _hook

if "antenv.axon_hooks" not in sys.modules:
    _hooks_mod = types.ModuleType("antenv.axon_hooks")
    _HOOK = _make_ntff_hook()
    _hooks_mod.get_axon_ntff_profile_hook = lambda: _HOOK
    _hooks_mod.set_axon_ntff_profile_hook = lambda h: None
    sys.modules["antenv.axon_hooks"] = _hooks_mod

import concourse.bass as bass
import concourse.tile as tile
from concourse import mybir
from concourse import bass_utils
bass_utils.upload_artifacts = lambda tmpdir: tmpdir  # no S3 in this container
from contextlib import ExitStack

F32 = mybir.dt.float32
BF16 = mybir.dt.bfloat16
F16 = mybir.dt.float16
AF = mybir.ActivationFunctionType
OP = mybir.AluOpType

NCORES = 8
CIN = 64        # model channels in
D = 128         # d_inner
NST = 16        # d_state
RANK = 4        # dt_rank
KCONV = 4
L = 4096
T = 512         # time tile for PSUM-bound ops
NT = L // T
CH = 1024       # n-loop chunk length
NCH = L // CH


def _legalize_sync_waits(nc):
    """Walrus codegen allows only one inline sync-wait per compute
    instruction; hoist surplus waits onto a preceding same-engine Drain."""
    SAFE = set()
    for f in nc.m.functions:
        for blk in f.blocks:
            insts = blk.instructions
            i = 0
            while i < len(insts):
                inst = insts[i]
                si = inst.sync_info
                if (si is not None and si.on_wait and len(si.on_wait) > 1
                        and inst.opcode not in SAFE):
                    waits = list(si.on_wait)
                    for w in waits[:-1]:
                        d = mybir.InstDrain(
                            name=nc.get_next_instruction_name(),
                            ins=[], outs=[], bass_is_fusable=False)
                        d.engine = inst.engine
                        d.sync_info = mybir.SyncInfo(on_wait=[w], on_update=[])
                        insts.insert(i, d)
                        i += 1
                    inst.sync_info = mybir.SyncInfo(
                        on_wait=[waits[-1]], on_update=list(si.on_update))
                    i += 1
                else:
                    i += 1


SIM_SAFE = False  # emit Silu as Identity+Sigmoid+mul so CoreSim can run it


def _emit_silu(nc, nlp, out_sl, in_ps, bias, blk_i, j, which):
    if not SIM_SAFE:
        if bias is None:
            nc.scalar.activation(out_sl, in_ps, AF.Silu)
        else:
            nc.scalar.activation(out_sl, in_ps, AF.Silu, bias=bias)
        return
    v = nlp.tile(list(in_ps.shape), F32, tag="lnt", name=f"sv_{which}_{blk_i}_{j}")
    if bias is None:
        nc.scalar.activation(v[:], in_ps, AF.Identity)
    else:
        nc.scalar.activation(v[:], in_ps, AF.Identity, bias=bias)
    s = nlp.tile(list(in_ps.shape), F32, tag="lnt2", name=f"ss_{which}_{blk_i}_{j}")
    nc.scalar.activation(s[:], v[:], AF.Sigmoid)
    nc.vector.tensor_mul(out_sl, v[:], s[:])


def _emit_block(nc, tc, ctx, pools, xpad, P, blk_i, out_final=None):
    """Emit one mamba block + layernorm + relu.

    xpad: SBUF [CIN, 3+L] fp32, first 3 cols zero.
    Returns x2pad tile (next block input) if out_final is None, else writes
    the permuted result into out_final.
    """
    const, big, nlp, psA, psB, psP = pools
    COUT = P["wout"].shape[1]   # 64 for block1, 128 for block2

    # ---- stage 1: xz matmuls (conv folded), silu ----
    xc = big.tile([D, L], F32, tag="xc", name=f"xc_{blk_i}")
    zs = big.tile([D, L], F16, tag="zs", name=f"zs_{blk_i}")
    for j in range(NT):
        ps_xc = psA.tile([D, T], F32, tag="mm", name=f"psxc_{blk_i}_{j}")
        for k in range(KCONV):
            nc.tensor.matmul(ps_xc[:], P["wk"][k][:], xpad[:, j*T + k : j*T + k + T],
                             start=(k == 0), stop=(k == KCONV - 1))
        _emit_silu(nc, nlp, xc[:, j*T:(j+1)*T], ps_xc[:], P["bconv"][:], blk_i, j, "xc")
        ps_z = psA.tile([D, T], F32, tag="mm", name=f"psz_{blk_i}_{j}")
        nc.tensor.matmul(ps_z[:], P["wz"][:], xpad[:, 3 + j*T : 3 + (j+1)*T],
                         start=True, stop=True)
        _emit_silu(nc, nlp, zs[:, j*T:(j+1)*T], ps_z[:], None, blk_i, j, "z")

    # ---- stage 2: proj = w_x^T xc -> dtr(4) B(16) C(16) rows ----
    proj = big.tile([RANK + 2*NST, L], F32, tag="proj", name=f"proj_{blk_i}")
    for j in range(NT):
        ps_p = psP.tile([RANK + 2*NST, T], F32, tag="proj", name=f"psp_{blk_i}_{j}")
        nc.tensor.matmul(ps_p[:], P["wx"][:], xc[:, j*T:(j+1)*T], start=True, stop=True)
        nc.scalar.copy(proj[:, j*T:(j+1)*T], ps_p[:])

    # ---- stage 3: dt = softplus(w_dt^T dtr + b_dt) = Ln(1 + Exp(.)) ----
    dt = big.tile([D, L], F32, tag="dt", name=f"dt_{blk_i}")
    for j in range(NT):
        ps_d = psA.tile([D, T], F32, tag="mm", name=f"psd_{blk_i}_{j}")
        nc.tensor.matmul(ps_d[:], P["wdt"][:], proj[0:RANK, j*T:(j+1)*T],
                         start=True, stop=True)
        e_t = nlp.tile([D, T], F32, tag="lnt", name=f"spe_{blk_i}_{j}")
        nc.scalar.activation(e_t[:], ps_d[:], AF.Exp, bias=P["bdt"][:])
        nc.scalar.activation(dt[:, j*T:(j+1)*T], e_t[:], AF.Ln, bias=const["one_d"][:])

    # ---- stage 4: dtxc = dt * xc; y_acc init = xc * D (skip path) ----
    dtxc = big.tile([D, L], F32, tag="dtxc", name=f"dtxc_{blk_i}")
    nc.vector.tensor_mul(dtxc[:], dt[:], xc[:])
    y_acc = big.tile([D, L], F16, tag="yacc", name=f"yacc_{blk_i}")
    nc.vector.tensor_scalar(y_acc[:], xc[:], P["D"][:], None, OP.mult)

    # ---- stage 5: per-state scan & accumulate y ----
    TBC = T  # bcast matmul tile
    for n in range(NST):
        brow = big.tile([1, L], F32, tag="brow", bufs=2, name=f"brow_{blk_i}_{n}")
        nc.sync.dma_start(brow[:], proj[RANK + n : RANK + n + 1, :])
        crow = big.tile([1, L], F32, tag="brow", bufs=2, name=f"crow_{blk_i}_{n}")
        nc.sync.dma_start(crow[:], proj[RANK + NST + n : RANK + NST + n + 1, :])
        hprev = None
        for c in range(NCH):
            c0 = c * CH
            dA = nlp.tile([D, CH], F32, tag="dA", name=f"dA_{blk_i}_{n}_{c}")
            nc.scalar.activation(dA[:], dt[:, c0:c0+CH], AF.Exp, scale=P["A"][:, n:n+1])
            dbx = nlp.tile([D, CH], F32, tag="dbx", name=f"dbx_{blk_i}_{n}_{c}")
            for j in range(CH // TBC):
                t0 = c0 + j * TBC
                ps_b = psB.tile([D, TBC], F32, tag="bc", name=f"psb_{blk_i}_{n}_{c}_{j}")
                nc.tensor.matmul(ps_b[:], const["ones1"][:], brow[0:1, t0:t0+TBC],
                                 start=True, stop=True)
                nc.vector.tensor_mul(dbx[:, j*TBC:(j+1)*TBC],
                                     dtxc[:, t0:t0+TBC], ps_b[:])
            h = nlp.tile([D, CH], F16, tag="h", name=f"h_{blk_i}_{n}_{c}")
            init = 0.0 if c == 0 else hprev[:, CH-1:CH]
            nc.vector.tensor_tensor_scan(h[:], dA[:], dbx[:], init, OP.mult, OP.add)
            hC = nlp.tile([D, CH], F16, tag="dbx", name=f"hC_{blk_i}_{n}_{c}")
            for j in range(CH // TBC):
                t0 = c0 + j * TBC
                ps_c = psB.tile([D, TBC], F32, tag="bc", name=f"psc_{blk_i}_{n}_{c}_{j}")
                nc.tensor.matmul(ps_c[:], const["ones1"][:], crow[0:1, t0:t0+TBC],
                                 start=True, stop=True)
                nc.vector.tensor_mul(hC[:, j*TBC:(j+1)*TBC],
                                     h[:, j*TBC:(j+1)*TBC], ps_c[:])
            nc.vector.tensor_add(y_acc[:, c0:c0+CH], y_acc[:, c0:c0+CH], hC[:])
            hprev = h

    # ---- stage 6: y = y_acc * zs (in place) ----
    nc.vector.tensor_mul(y_acc[:], y_acc[:], zs[:])

    # ---- stage 7: out matmul + layernorm stats ----
    y1 = big.tile([COUT, L], F32, tag="proj", name=f"y1_{blk_i}")
    mu = big.tile([1, L], F32, tag="dt", name=f"mu_{blk_i}")
    musq = big.tile([1, L], F32, tag="musq", name=f"musq_{blk_i}")
    for j in range(NT):
        ps_y = psA.tile([COUT, T], F32, tag="mm", name=f"psy_{blk_i}_{j}")
        nc.tensor.matmul(ps_y[:], P["wout"][:], y_acc[:, j*T:(j+1)*T],
                         start=True, stop=True)
        nc.scalar.copy(y1[:, j*T:(j+1)*T], ps_y[:])
        y1sq = nlp.tile([COUT, T], F32, tag="lnt", name=f"y1sq_{blk_i}_{j}")
        nc.scalar.activation(y1sq[:], ps_y[:], AF.Square)
        ps_m = psP.tile([1, T], F32, tag="stat", name=f"psm_{blk_i}_{j}")
        nc.tensor.matmul(ps_m[:], P["onesc"][:], y1[:, j*T:(j+1)*T],
                         start=True, stop=True)
        nc.scalar.copy(mu[:, j*T:(j+1)*T], ps_m[:])
        ps_m2 = psP.tile([1, T], F32, tag="stat", name=f"psm2_{blk_i}_{j}")
        nc.tensor.matmul(ps_m2[:], P["onesc"][:], y1sq[:], start=True, stop=True)
        nc.scalar.copy(musq[:, j*T:(j+1)*T], ps_m2[:])
    # var = musq - mu^2 (in place), rstd = exp(-0.5*ln(var+eps)) (in place)
    for j in range(NT):
        sl = slice(j*T, (j+1)*T)
        ps_s = psP.tile([1, T], F32, tag="stat", name=f"pss_{blk_i}_{j}")
        nc.scalar.activation(ps_s[:], mu[:, sl], AF.Square)
        nc.vector.tensor_sub(musq[:, sl], musq[:, sl], ps_s[:])
        nc.scalar.activation(musq[:, sl], musq[:, sl], AF.Ln, bias=const["eps"][:])
        nc.scalar.activation(musq[:, sl], musq[:, sl], AF.Exp, scale=-0.5)

    # apply: out = relu(g*(y1 - mu)*rstd + b)
    if out_final is None:
        x2pad = big.tile([COUT, 3 + L], F32, tag="xpad", name=f"x2pad_{blk_i}")
        nc.vector.memset(x2pad[:, 0:3], 0.0)
    for j in range(NT):
        ps_mb = psA.tile([COUT, T], F32, tag="mm", name=f"psmb_{blk_i}_{j}")
        nc.tensor.matmul(ps_mb[:], P["onesr"][:], mu[:, j*T:(j+1)*T],
                         start=True, stop=True)
        ps_rb = psA.tile([COUT, T], F32, tag="mm", name=f"psrb_{blk_i}_{j}")
        nc.tensor.matmul(ps_rb[:], P["onesr"][:], musq[:, j*T:(j+1)*T],
                         start=True, stop=True)
        t1 = nlp.tile([COUT, T], F32, tag="lnt", name=f"lnt1_{blk_i}_{j}")
        nc.vector.tensor_sub(t1[:], y1[:, j*T:(j+1)*T], ps_mb[:])
        t2 = nlp.tile([COUT, T], F32, tag="lnt2", name=f"lnt2_{blk_i}_{j}")
        nc.vector.tensor_mul(t2[:], t1[:], ps_rb[:])
        if out_final is None:
            nc.scalar.activation(x2pad[:, 3 + j*T : 3 + (j+1)*T], t2[:], AF.Relu,
                                 bias=P["bln"][:], scale=P["gln"][:])
        else:
            in_v = t2[:].rearrange("p (h w) -> p h w", w=64)
            out_v = out_final[:].rearrange("p (w h) -> p h w", h=64)[:, 8*j:8*(j+1), :]
            nc.scalar.activation(out_v, in_v, AF.Relu,
                                 bias=P["bln"][:], scale=P["gln"][:])
    return None if out_final is not None else x2pad


def build_nc(legalize=True, sim_safe=False):
    global SIM_SAFE
    SIM_SAFE = sim_safe
    nc = bass.Bass("TRN2", debug=False)
    f32 = np.float32

    def din(name, shape, dt=F32):
        return nc.dram_tensor(name, list(shape), dt, kind="ExternalInput")

    x_d = din("x", (CIN, L))
    ins = {}
    for b in (1, 2):
        ins[f"wk{b}"] = [din(f"wk{b}_{k}", (CIN, D)) for k in range(KCONV)]
        ins[f"wz{b}"] = din(f"wz{b}", (CIN, D))
        ins[f"bconv{b}"] = din(f"bconv{b}", (D, 1))
        ins[f"wx{b}"] = din(f"wx{b}", (D, RANK + 2*NST))
        ins[f"wdt{b}"] = din(f"wdt{b}", (RANK, D))
        ins[f"bdt{b}"] = din(f"bdt{b}", (D, 1))
        ins[f"A{b}"] = din(f"A{b}", (D, NST))
        ins[f"D{b}"] = din(f"D{b}", (D, 1))
        cout = CIN if b == 1 else 2 * CIN
        ins[f"wout{b}"] = din(f"wout{b}", (D, cout), F16)
        ins[f"gln{b}"] = din(f"gln{b}", (cout, 1))
        ins[f"bln{b}"] = din(f"bln{b}", (cout, 1))
        ins[f"onesc{b}"] = din(f"onesc{b}", (cout, 1))   # 1/cout for mean
        ins[f"onesr{b}"] = din(f"onesr{b}", (1, cout))   # ones row for bcast
    ins["ones1"] = din("ones1", (1, D))
    ins["one_d"] = din("one_d", (D, 1))
    ins["eps"] = din("eps", (1, 1))
    out_d = nc.dram_tensor("out", [2*CIN, L], F32, kind="ExternalOutput")

    with tile.TileContext(nc) as tc:
        with ExitStack() as ctx:
            cpool = ctx.enter_context(tc.tile_pool(name="const", bufs=1))
            big = ctx.enter_context(tc.tile_pool(name="big", bufs=1))
            nlp = ctx.enter_context(tc.tile_pool(name="nloop", bufs=2))
            psA = ctx.enter_context(tc.tile_pool(name="psA", bufs=2, space="PSUM"))
            psB = ctx.enter_context(tc.tile_pool(name="psB", bufs=4, space="PSUM"))
            psP = ctx.enter_context(tc.tile_pool(name="psP", bufs=1, space="PSUM"))

            def load(name, dram):
                t = cpool.tile(list(dram.shape), dram.dtype, tag=name, name=name)
                nc.sync.dma_start(t[:], dram.ap())
                return t

            const = {"ones1": load("ones1", ins["ones1"]),
                     "one_d": load("one_d", ins["one_d"]),
                     "eps": load("eps", ins["eps"])}
            P = {}
            for b in (1, 2):
                cout = CIN if b == 1 else 2 * CIN
                P[b] = {
                    "wk": [load(f"wk{b}_{k}", ins[f"wk{b}"][k]) for k in range(KCONV)],
                    "wz": load(f"wz{b}", ins[f"wz{b}"]),
                    "bconv": load(f"bconv{b}", ins[f"bconv{b}"]),
                    "wx": load(f"wx{b}", ins[f"wx{b}"]),
                    "wdt": load(f"wdt{b}", ins[f"wdt{b}"]),
                    "bdt": load(f"bdt{b}", ins[f"bdt{b}"]),
                    "A": load(f"A{b}", ins[f"A{b}"]),
                    "D": load(f"D{b}", ins[f"D{b}"]),
                    "wout": load(f"wout{b}", ins[f"wout{b}"]),
                    "gln": load(f"gln{b}", ins[f"gln{b}"]),
                    "bln": load(f"bln{b}", ins[f"bln{b}"]),
                    "onesc": load(f"onesc{b}", ins[f"onesc{b}"]),
                    "onesr": load(f"onesr{b}", ins[f"onesr{b}"]),
                }

            xpad = big.tile([CIN, 3 + L], F32, tag="xpad")
            nc.vector.memset(xpad[:, 0:3], 0.0)
            nc.sync.dma_start(xpad[:, 3:], x_d.ap())

            out_sb = big.tile([2*CIN, L], F32, tag="dtxc")  # dtxc dead by then
            pools = (const, big, nlp, psA, psB, psP)
            x2pad = _emit_block(nc, tc, ctx, pools, xpad, P[1], 1, out_final=None)
            _emit_block(nc, tc, ctx, pools, x2pad, P[2], 2, out_final=out_sb)
            nc.sync.dma_start(out_d.ap(), out_sb[:])

    if legalize:
        _legalize_sync_waits(nc)
    return nc


_NC_CACHE = {}
_LAST_EXEC_NS = {}

def _get_nc():
    if "nc" not in _NC_CACHE:
        _NC_CACHE["nc"] = build_nc()
    return _NC_CACHE["nc"]


def _host_params(inputs):
    """Fold conv into input projection; compute derived tensors."""
    f32 = np.float32
    maps = {}
    for b in (1, 2):
        w_in = np.asarray(inputs[f"w_in{b}"], f32)       # (64, 256)
        w_conv = np.asarray(inputs[f"w_conv{b}"], f32)   # (128, 4)
        cout = CIN if b == 1 else 2 * CIN
        for k in range(KCONV):
            maps[f"wk{b}_{k}"] = np.ascontiguousarray(w_in[:, :D] * w_conv[:, k][None, :])
        maps[f"wz{b}"] = np.ascontiguousarray(w_in[:, D:])
        maps[f"bconv{b}"] = np.asarray(inputs[f"b_conv{b}"], f32).reshape(D, 1)
        maps[f"wx{b}"] = np.asarray(inputs[f"w_x{b}"], f32)
        maps[f"wdt{b}"] = np.asarray(inputs[f"w_dt{b}"], f32)
        maps[f"bdt{b}"] = np.asarray(inputs[f"b_dt{b}"], f32).reshape(D, 1)
        maps[f"A{b}"] = -np.exp(np.asarray(inputs[f"A_log{b}"], f32))
        maps[f"D{b}"] = np.asarray(inputs[f"D{b}"], f32).reshape(D, 1)
        maps[f"wout{b}"] = np.asarray(inputs[f"w_out{b}"], f32).astype(np.float16)
        maps[f"gln{b}"] = np.asarray(inputs[f"g_ln{b}"], f32).reshape(cout, 1)
        maps[f"bln{b}"] = np.asarray(inputs[f"b_ln{b}"], f32).reshape(cout, 1)
        maps[f"onesc{b}"] = np.full((cout, 1), 1.0 / cout, f32)
        maps[f"onesr{b}"] = np.ones((1, cout), f32)
    maps["ones1"] = np.ones((1, D), f32)
    maps["one_d"] = np.ones((D, 1), f32)
    maps["eps"] = np.full((1, 1), 1e-5, f32)
    return maps


def kernel(**inputs, ):
    return _run(inputs, trace=False)


def _run(inputs, trace=False):
    nc = _get_nc()
    x = np.asarray(inputs["x"], np.float32)              # (8, 64, 64, 64)
    b, c, hh, ww = x.shape
    params = _host_params(inputs)
    in_maps = []
    for i in range(NCORES):
        m = dict(params)
        m["x"] = np.ascontiguousarray(x[i].reshape(c, hh * ww))
        in_maps.append(m)
    res = bass_utils.run_bass_kernel_spmd(nc, in_maps, core_ids=list(range(NCORES)),
                                          trace=trace)
    if trace:
        _LAST_EXEC_NS["ns"] = res.exec_time_ns
        _LAST_EXEC_NS["res"] = res
    out = np.stack([res.results[i]["out"] for i in range(NCORES)])
    return out.reshape(b, 2 * c, ww, hh).astype(np.float32)
